# revision 1
# baseline (speedup 1.0000x reference)
"""LIF fully-connected neuron layer on 8 Trainium2 NeuronCores.

reference semantics (per sample b, hidden unit h):
    x[b,t,h] = sum_d input[b,t,d] * W[h,d] + bias[h]
    m_t   = mem_{t-1} + x_t
    spike = m_t > THRESH
    mem_t = m_t * (1-spike) * DECAY
    out[b,t,h] = spike

Strategy:
  - Data-parallel over batch: core c handles samples [8c, 8c+8).
  - Host pre-transposes input to [d, t, b] so matmul operands load naturally
    (contraction dim d on partitions) -- zero on-device transposes.
  - Matmul: out[h_tile][128 h, (t,b)] accumulated over 8 k-tiles in PSUM,
    fp32, N=256 (32 timesteps x 8 samples) per window.
  - ScalarE copies PSUM->SBUF with per-partition bias add (Identity act).
  - Scan: one fused custom DVE op per timestep over [128, 64] lanes
    (lane = (h_tile, b), partition = h_lo):
        u' = m * (m <= TH),  m = u*DECAY + x_t
    u' is the pre-decay post-reset membrane; spikes are derived in 64-step
    batches as (u' == 0) and DMA'd out.  (u'==0 with no spike requires the
    membrane to be exactly 0.0 -- measure-zero, verified empirically.)
  - Host reassembles [B, T, H] from the device layout.
"""

import numpy as np

# ---- problem constants (hardcoded per contest contract) ----
B, T, D, H = 64, 512, 1024, 1024
N_CORES = 8
B_L = B // N_CORES            # 8 samples per core
P = 128                       # partitions
DT, HT = D // P, H // P       # 8 k-tiles, 8 h-tiles
WT = 32                       # timesteps per matmul window
NW = T // WT                  # 16 windows
NCOL = WT * B_L               # 256 moving columns per window
F = HT * B_L                  # 64 scan lanes in free dim
BLK = 64                      # timesteps per spike/output block
NB = T // BLK                 # 8 output blocks

DECAY = 200.0 / 255.0
THRESH = 0.3

_CACHE = {}


def _register_lif_op():
    from concourse.dve_spec import Spec, Src0, Src1, C0, C1, lower
    from concourse.dve_ops import (
        DveOp, OPS, CUSTOM_DVE_SPECS, _SUB_OPCODE_FOR_NAME, _CUSTOM_DVE_ROW_BASE,
    )
    from concourse.dve_uop import DveOpSpec

    name = "LIF_STEP_ANT"
    for op in OPS:
        if op.name == name:
            return op

    m = Src0 * C0 + Src1
    body = (m <= C1) * m

    def ref(in0, in1, s0, s1, imm2):
        mm = (in0 * np.float32(s0) + in1).astype(np.float32)
        return (mm * (mm <= np.float32(s1))).astype(np.float32)

    spec = Spec(body=body, reference=ref)
    opcode = _CUSTOM_DVE_ROW_BASE + len(OPS)
    shas = {}
    for ver in ("v3", "v4"):
        uops = lower(spec, ver=ver)
        shas[ver] = DveOpSpec(name=name, opcode=opcode, uops=uops, rd1_en=True).sha(ver)
    op = DveOp(name, spec, subdim=False, uops_sha=shas)
    OPS.append(op)
    _SUB_OPCODE_FOR_NAME[name] = opcode
    CUSTOM_DVE_SPECS[name] = spec
    return op


def _build():
    if "nc" in _CACHE:
        return _CACHE["nc"]
    from contextlib import ExitStack
    import concourse.bacc as bacc
    import concourse.tile as tile
    from concourse import mybir

    lif_op = _register_lif_op()

    nc = bacc.Bacc("TRN2", target_bir_lowering=False, debug=False,
                   num_devices=N_CORES)
    f32 = mybir.dt.float32
    xin_d = nc.dram_tensor("xin", [D, T * B_L], f32, kind="ExternalInput").ap()
    wt_d = nc.dram_tensor("wt", [D, H], f32, kind="ExternalInput").ap()
    bias_d = nc.dram_tensor("bias", [P, HT], f32, kind="ExternalInput").ap()
    out_d = nc.dram_tensor("out", [NB, P, BLK * F], f32, kind="ExternalOutput").ap()

    with tile.TileContext(nc) as tc, ExitStack() as ctx:
        const_pool = ctx.enter_context(tc.tile_pool(name="const", bufs=1))
        rhs_pool = ctx.enter_context(tc.tile_pool(name="rhs", bufs=3))
        xs_pool = ctx.enter_context(tc.tile_pool(name="xs", bufs=2))
        psum_pool = ctx.enter_context(tc.tile_pool(name="psum", bufs=2, space="PSUM"))
        spk_pool = ctx.enter_context(tc.tile_pool(name="spk", bufs=2))

        # --- constants ---
        wt_s = const_pool.tile([P, DT * H], f32)          # [d_lo, (dt, h)]
        nc.sync.dma_start(
            wt_s[:].rearrange("p (dt h) -> p dt h", dt=DT),
            wt_d.rearrange("(dt p) h -> p dt h", dt=DT),
        )
        bias_s = const_pool.tile([P, HT], f32)
        nc.sync.dma_start(bias_s[:], bias_d)

        # --- membrane ring: 128 slots of F lanes; slot t%128 = u after step t
        ring = const_pool.tile([P, 2 * BLK * F], f32)
        nc.vector.memset(ring[:, (2 * BLK - 1) * F:], 0.0)

        xin_r = xin_d.rearrange("(dt p) n -> p dt n", dt=DT)

        for w in range(NW):
            # load input^T window: [d_lo, (dt, 32t x 8b)]  (1 MiB)
            rhs = rhs_pool.tile([P, DT * NCOL], f32)
            nc.sync.dma_start(
                rhs[:].rearrange("p (dt n) -> p dt n", dt=DT),
                xin_r[:, :, w * NCOL:(w + 1) * NCOL],
            )
            # matmul: 8 h-tiles x 8 k-steps, PSUM fp32
            pt = [psum_pool.tile([P, 2 * NCOL], f32, tag=f"g{g}", name=f"pt{g}")
                  for g in range(4)]
            for ht in range(HT):
                acc = pt[ht // 2][:, (ht % 2) * NCOL:(ht % 2 + 1) * NCOL]
                for dt in range(DT):
                    nc.tensor.matmul(
                        acc,
                        wt_s[:, dt * H + ht * P: dt * H + ht * P + P],
                        rhs[:, dt * NCOL:(dt + 1) * NCOL],
                        start=(dt == 0),
                        stop=(dt == DT - 1),
                    )
            # PSUM -> SBUF with bias add (ScalarE)
            xs = xs_pool.tile([P, HT * NCOL], f32)        # [p, (ht, t32, b8)]
            for ht in range(HT):
                nc.scalar.activation(
                    xs[:, ht * NCOL:(ht + 1) * NCOL],
                    pt[ht // 2][:, (ht % 2) * NCOL:(ht % 2 + 1) * NCOL],
                    mybir.ActivationFunctionType.Identity,
                    bias=bias_s[:, ht:ht + 1],
                    scale=1.0,
                )
            # scan: one fused DVE op per timestep
            xs_r = xs[:].rearrange("p (ht t b) -> p t ht b", ht=HT, t=WT, b=B_L)
            for tt in range(WT):
                t = w * WT + tt
                s_out = (t % (2 * BLK)) * F
                s_in = ((t - 1) % (2 * BLK)) * F
                nc.vector._custom_dve(
                    lif_op,
                    out=ring[:, s_out:s_out + F],
                    in0=ring[:, s_in:s_in + F],
                    in1=xs_r[:, tt],
                    s0=DECAY,
                    s1=THRESH,
                )
            # every 2 windows: derive spikes for the finished 64-step block
            if w % 2 == 1:
                blk = w // 2
                half = (blk % 2) * BLK * F
                spk = spk_pool.tile([P, BLK * F], f32)
                nc.vector.tensor_scalar(
                    out=spk[:], in0=ring[:, half:half + BLK * F],
                    scalar1=0.0, scalar2=None, op0=mybir.AluOpType.is_equal,
                )
                nc.sync.dma_start(out_d[blk], spk[:])

    nc.compile()
    _CACHE["nc"] = nc
    return nc


def kernel(input_data, W, b):
    from concourse.bass_utils import run_bass_kernel_spmd

    input_data = np.asarray(input_data, dtype=np.float32)
    W = np.asarray(W, dtype=np.float32)
    b = np.asarray(b, dtype=np.float32)

    nc = _build()

    wt = np.ascontiguousarray(W.T)                       # [d, h]
    bias = np.ascontiguousarray(b.reshape(HT, P).T)      # [h_lo, ht]
    in_maps = []
    for c in range(N_CORES):
        xc = input_data[c * B_L:(c + 1) * B_L]           # [8, T, D]
        xin = np.ascontiguousarray(xc.transpose(2, 1, 0)).reshape(D, T * B_L)
        in_maps.append({"xin": xin, "wt": wt, "bias": bias})

    res = run_bass_kernel_spmd(nc, in_maps, core_ids=list(range(N_CORES)))

    outs = []
    for c in range(N_CORES):
        o = res.results[c]["out"]                        # [NB, P, BLK*F]
        o = o.reshape(NB, P, BLK, HT, B_L)               # [blk, h_lo, t, ht, b]
        o = o.transpose(4, 0, 2, 3, 1).reshape(B_L, T, H)
        outs.append(o)
    return np.ascontiguousarray(np.concatenate(outs, axis=0))

